# revision 3
# baseline (speedup 1.0000x reference)
"""Multi-head attention (B=2, S=2048, E=1024, H=16, D=64) on 8 Trainium2 NeuronCores.

Sharding (tensor-parallel over heads x data-parallel over batch): core c ->
batch b=c//4, head-group g=c%4 (4 heads per core).  Each core runs the full
per-group pipeline; the host sums the 4 partial out-projections per batch.

Per-core schedule (bf16 x/weights/es/v/a, f32r q/k, fp32 PSUM):
  head:  k-proj (all q-blocks), v-proj st0-7, q-proj qb0 -- accumulation
         chains over SBUF-resident bf16 xT, biases folded into the PSUM->SBUF
         copies (per-partition tensor_scalar for q/k, broadcast tiles for v/y).
  P2:    per (head-pair, qb): scores k-tile one step ahead of PV; exp on ACT
         (bias -3 shift, softmax-invariant); ones-column in V accumulates the
         softmax denominators.  Remaining projections, out-projection chains,
         and division multiplies are woven between the kt iterations as
         fine-grained fillers to keep the PE dense while ACT streams exps.
  divisions: po copied to SBUF (frees PSUM), reciprocal + DRAM-bounce
         broadcast mid-loop (latency hidden), PE ones-matmul broadcast at the
         tail (no DMA latency on the critical path).
  P3:    folded into the pr=1 weave; tail drains the last q-block.

Input DMAs are issued in need-order on both HWDGE queues (SP: weights,
Activation: xT slices + half the y write-backs).
"""

import contextlib
from collections import deque

import numpy as np
import ml_dtypes
import concourse.bass as bass
import concourse.tile as tile
from concourse import bacc, mybir
from concourse.bass_utils import run_bass_kernel_spmd

F32 = mybir.dt.float32
F32R = mybir.dt.float32r
BF16 = mybir.dt.bfloat16

S = 2048
E = 1024
HL = 4
D = 64
QB = 512
NQB = S // QB          # 4
NKT = S // 128         # 16
NST = S // 128         # 16
NKC = E // 128         # 8
HV = 65                # per-head v width: 64 d + 1 ones column
EXP = mybir.ActivationFunctionType.Exp

_CACHE = {}


def _build(repeat=1):
    nc = bacc.Bacc("TRN2", target_bir_lowering=False, debug=False, num_devices=8)

    xT_d = nc.dram_tensor("xT", [E, S], BF16, kind="ExternalInput").ap()
    wqk_d = nc.dram_tensor("wqk", [E, 512], BF16, kind="ExternalInput").ap()
    bqk_d = nc.dram_tensor("bqk", [512, 1], F32, kind="ExternalInput").ap()
    wv_d = nc.dram_tensor("wv", [E, HL * HV], BF16, kind="ExternalInput").ap()
    bv_d = nc.dram_tensor("bv", [1, HL * HV], F32, kind="ExternalInput").ap()
    wo_d = nc.dram_tensor("wo", [256, E], BF16, kind="ExternalInput").ap()
    bo_d = nc.dram_tensor("bo", [1, E], F32, kind="ExternalInput").ap()
    nb3_d = nc.dram_tensor("nb3", [128, 1], F32, kind="ExternalInput").ap()
    ones_d = nc.dram_tensor("ones", [1, 64], F32R, kind="ExternalInput").ap()
    y_d = nc.dram_tensor("y", [S, E], BF16, kind="ExternalOutput").ap()
    den_d = nc.dram_tensor("den_scratch", [2 * HL * NQB, QB], F32)

    with tile.TileContext(nc) as tc:
        with (
            tc.tile_pool(name="wpool", bufs=1) as wpool,
            tc.tile_pool(name="xtp", bufs=1) as xtp,
            tc.tile_pool(name="big", bufs=1) as big,
            tc.tile_pool(name="espool", bufs=4) as espool,
            tc.tile_pool(name="denpool", bufs=4) as denpool,
            tc.tile_pool(name="pospool", bufs=4) as pospool,
            tc.tile_pool(name="bcpool", bufs=4) as bcpool,
            tc.tile_pool(name="ypool", bufs=3) as ypool,
            tc.tile_pool(name="pp", bufs=2, space="PSUM") as pp,
            tc.tile_pool(name="pq", bufs=3, space="PSUM") as pq,
            tc.tile_pool(name="pj", bufs=1, space="PSUM") as pj,
        ):
            loop = tc.For_i(0, repeat) if repeat > 1 else contextlib.nullcontext()
            with loop:
                wqk_t = [wpool.tile([128, 512], BF16, name=f"wqk{k}")
                         for k in range(NKC)]
                wv_t = [wpool.tile([128, HL * HV], BF16, name=f"wv{k}")
                        for k in range(NKC)]
                wo_t = [wpool.tile([128, E], BF16, name=f"wo{c}") for c in range(2)]
                bq_t = [wpool.tile([128, 1], F32, name=f"bq{m}") for m in range(4)]
                bv_bc = wpool.tile([128, HL * HV], F32, name="bv_bc")
                bo_bc = wpool.tile([128, E], F32, name="bo_bc")
                nb3 = wpool.tile([128, 1], F32, name="nb3")
                nc.sync.dma_start(nb3[:], nb3_d[:])
                ones64 = wpool.tile([1, 64], F32R, name="ones64")
                nc.sync.dma_start(ones64[:], ones_d[:])
                xt = [xtp.tile([128, S], BF16, name=f"xt{k}") for k in range(NKC)]

                for k in range(NKC):
                    nc.sync.dma_start(wqk_t[k][:], wqk_d[k * 128:(k + 1) * 128, :])
                    nc.scalar.dma_start(xt[k][:, 0:QB],
                                        xT_d[k * 128:(k + 1) * 128, 0:QB])
                for m in range(4):
                    nc.sync.dma_start(bq_t[m][:], bqk_d[m * 128:(m + 1) * 128, :])
                nc.sync.dma_start(bv_bc[:], bv_d[0:1, :].to_broadcast((128, HL * HV)))
                for k in range(NKC):
                    nc.sync.dma_start(wv_t[k][:], wv_d[k * 128:(k + 1) * 128, :])
                for qb in range(1, NQB):
                    sl = slice(qb * QB, (qb + 1) * QB)
                    for k in range(NKC):
                        nc.scalar.dma_start(xt[k][:, sl],
                                            xT_d[k * 128:(k + 1) * 128, sl])
                for c in range(2):
                    nc.sync.dma_start(wo_t[c][:], wo_d[c * 128:(c + 1) * 128, :])
                nc.sync.dma_start(bo_bc[:], bo_d[0:1, :].to_broadcast((128, E)))

                qkT = [big.tile([128, S], F32R, name=f"qkT{m}") for m in range(4)]
                vt = [big.tile([128, HL, HV], BF16, name=f"vt{st}")
                      for st in range(NST)]
                a_t = [big.tile([128, S], BF16, name=f"a{c}") for c in range(2)]

                TAG = {id(pp): "pp", id(pq): "pq", id(pj): "pj"}

                def qk_chain_units(m, qb, pool):
                    sl = slice(qb * QB, (qb + 1) * QB)
                    state = {}

                    def mk(k0):
                        def u():
                            if "p" not in state:
                                state["p"] = pool.tile([128, QB], F32, name="prj",
                                                       tag=TAG[id(pool)])
                            p = state["p"]
                            for k in (k0, k0 + 1):
                                nc.tensor.matmul(
                                    p[:], wqk_t[k][:, m * 128:(m + 1) * 128],
                                    xt[k][:, sl], start=(k == 0),
                                    stop=(k == NKC - 1))
                            if k0 + 2 == NKC:
                                nc.vector.tensor_scalar_add(qkT[m][:, sl], p[:],
                                                            bq_t[m][:])
                        return u
                    return [mk(k0) for k0 in range(0, NKC, 2)]

                def v_chain_units(st, pool):
                    sl = slice(st * 128, (st + 1) * 128)
                    state = {}

                    def mk(k0):
                        def u():
                            if "p" not in state:
                                state["p"] = pool.tile([128, HL * HV], F32,
                                                       name="vprj",
                                                       tag=TAG[id(pool)])
                            p = state["p"]
                            for k in range(k0, k0 + 4):
                                nc.tensor.matmul(p[:], xt[k][:, sl], wv_t[k][:],
                                                 start=(k == 0),
                                                 stop=(k == NKC - 1))
                            if k0 + 4 == NKC:
                                dst = vt[st][:, :, :].rearrange("p h d -> p (h d)")
                                nc.vector.tensor_add(dst, p[:], bv_bc[:])
                        return u
                    return [mk(0), mk(4)]

                def p3_units(st, n, pool):
                    ssl = slice(st * 128, (st + 1) * 128)
                    nsl = slice(n * QB, (n + 1) * QB)
                    state = {}

                    def u1():
                        state["p"] = pool.tile([128, QB], F32, name="p3",
                                               tag=TAG[id(pool)])
                        nc.tensor.matmul(state["p"][:], a_t[0][:, ssl],
                                         wo_t[0][:, nsl], start=True, stop=False)

                    def u2():
                        p = state["p"]
                        nc.tensor.matmul(p[:], a_t[1][:, ssl], wo_t[1][:, nsl],
                                         start=False, stop=True)
                        yt = ypool.tile([128, QB], BF16, name="yt")
                        nc.vector.tensor_add(yt[:], p[:], bo_bc[:, nsl])
                        eng = nc.sync if (st + n) % 2 else nc.scalar
                        eng.dma_start(y_d[ssl, nsl], yt[:])
                    return [u1, u2]

                def division_units(pr, qb, po_a, po_b, tail=False):
                    pos = []
                    for idx, po_t in ((0, po_a), (1, po_b)):
                        po_s = pospool.tile([65, QB], F32, name="po_s")
                        nc.vector.tensor_copy(po_s[:], po_t[0:65, :])
                        pos.append(po_s)
                    units = []
                    for idx, po_s in enumerate(pos):
                        j = 2 * pr + idx
                        if tail:
                            den_r = denpool.tile([1, QB], F32R, name="den_r")
                            with nc.allow_low_precision(reason="f32r==f32"):
                                nc.vector.reciprocal(den_r[:], po_s[64:65, :])
                            bc = pq.tile([64, QB], F32, name="bct", tag="pq")
                            nc.tensor.matmul(bc[:], ones64[:], den_r[:],
                                             start=True, stop=True)
                        else:
                            den_r = denpool.tile([1, QB], F32, name="den_r")
                            nc.vector.reciprocal(den_r[:], po_s[64:65, :])
                            slot = (pr * HL * NQB) + idx * NQB + qb
                            nc.sync.dma_start(den_d[slot:slot + 1, :], den_r[:])
                            bc = bcpool.tile([64, QB], F32, name="bc")
                            nc.sync.dma_start(
                                bc[:],
                                den_d[slot:slot + 1, :].to_broadcast((64, QB)))

                        def unit(j=j, po_s=po_s, bc=bc):
                            a_out = a_t[j // 2][(j % 2) * 64:(j % 2) * 64 + 64,
                                                qb * QB:(qb + 1) * QB]
                            nc.vector.tensor_mul(a_out, po_s[0:64, :], bc[:])
                        units.append(unit)
                    return units

                def run_chain(units):
                    for u in units:
                        u()

                # ---- P1a head ------------------------------------------
                run_chain(qk_chain_units(2, 0, pp))
                run_chain(qk_chain_units(3, 0, pj))
                run_chain(v_chain_units(0, pq))
                run_chain(v_chain_units(1, pq))
                run_chain(qk_chain_units(2, 1, pp))
                run_chain(qk_chain_units(3, 1, pj))
                run_chain(qk_chain_units(0, 0, pp))
                run_chain(qk_chain_units(1, 0, pj))
                run_chain(v_chain_units(2, pq))
                run_chain(v_chain_units(3, pq))
                run_chain(qk_chain_units(2, 2, pp))
                run_chain(qk_chain_units(3, 2, pj))
                run_chain(v_chain_units(4, pp))
                run_chain(v_chain_units(5, pj))
                run_chain(qk_chain_units(2, 3, pp))
                run_chain(qk_chain_units(3, 3, pj))
                run_chain(v_chain_units(6, pp))
                run_chain(v_chain_units(7, pj))

                fillers = deque()

                def emit_qb(pr, qb, budget, tail=False):
                    j0, j1 = 2 * pr, 2 * pr + 1
                    qT = qkT[pr]
                    kT = qkT[2 + pr]
                    qsl = slice(qb * QB, (qb + 1) * QB)
                    po_a = pq.tile([HV, QB], F32, name="po_a", tag="pq")
                    po_b = pq.tile([HV, QB], F32, name="po_b", tag="pq")
                    es_t = {}

                    def scores(kt):
                        ps_t = pp.tile([128, 2 * QB], F32, name="ps_t", tag="pp")
                        ksl = slice(kt * 128, (kt + 1) * 128)
                        nc.tensor.matmul(ps_t[:, 0:QB], kT[0:64, ksl],
                                         qT[0:64, qsl], start=True, stop=True)
                        nc.tensor.matmul(ps_t[:, QB:2 * QB], kT[64:128, ksl],
                                         qT[64:128, qsl], start=True, stop=True)
                        es_t[kt] = espool.tile([128, 2 * QB], BF16, name="es")
                        nc.scalar.activation(es_t[kt], ps_t[:], EXP, bias=nb3[:])

                    scores(0)
                    for kt in range(NKT):
                        if kt + 1 < NKT:
                            scores(kt + 1)
                        for _ in range(budget[kt % len(budget)]):
                            if fillers:
                                fillers.popleft()()
                        es = es_t.pop(kt)
                        nc.tensor.matmul(po_a[:], vt[kt][:, j0, :], es[:, 0:QB],
                                         start=(kt == 0), stop=(kt == NKT - 1))
                        nc.tensor.matmul(po_b[:], vt[kt][:, j1, :],
                                         es[:, QB:2 * QB],
                                         start=(kt == 0), stop=(kt == NKT - 1))
                    return division_units(pr, qb, po_a, po_b, tail=tail)

                # qb0 weave: v st8..15 then q-proj(qb1)
                for st in range(8, 16):
                    fillers.extend(v_chain_units(st, pj))
                fillers.extend(qk_chain_units(0, 1, pj))
                fillers.extend(qk_chain_units(1, 1, pj))

                div_prev = emit_qb(0, 0, budget=[2, 1])

                for qb in range(1, NQB):        # pr=0, qb 1-3
                    if qb + 1 < NQB:
                        fillers.extend(qk_chain_units(0, qb + 1, pj))
                        fillers.extend(qk_chain_units(1, qb + 1, pj))
                    fillers.extend(div_prev)
                    div_prev = emit_qb(0, qb, budget=[1])

                for qb in range(NQB):           # pr=1
                    fillers.extend(div_prev)
                    if qb >= 1:
                        for st in range(4 * (qb - 1), 4 * qb):
                            for n in range(2):
                                fillers.extend(p3_units(st, n, pj))
                    div_prev = emit_qb(1, qb, budget=[3, 1, 2, 1],
                                       tail=(qb == NQB - 1))

                # ---- tail ----------------------------------------------
                while fillers:
                    fillers.popleft()()
                last_p3 = [p3_units(st, n, pp if (2 * i + n) % 2 else pj)
                           for i, st in enumerate(range(4 * (NQB - 1), 4 * NQB))
                           for n in range(2)]
                for units in last_p3:
                    units[0]()
                for u in div_prev:
                    u()
                for units in last_p3:
                    units[1]()

    nc.compile()
    return nc


def _shard_inputs(query, W_qkv, b_qkv, W_out, b_out):
    scale = np.float32(1.0 / np.sqrt(D))
    query = np.asarray(query, dtype=np.float32)
    W_qkv = np.asarray(W_qkv, dtype=np.float32)
    b_qkv = np.asarray(b_qkv, dtype=np.float32)
    W_out = np.asarray(W_out, dtype=np.float32)
    b_out = np.asarray(b_out, dtype=np.float32)

    W_q, W_k, W_v = W_qkv[:, :E], W_qkv[:, E:2 * E], W_qkv[:, 2 * E:]
    b_q, b_k, b_v = b_qkv[:E], b_qkv[E:2 * E], b_qkv[2 * E:]

    in_maps = []
    for c in range(8):
        b = c // 4
        g = c % 4
        hsl = slice(4 * g * D, (4 * g + 4) * D)
        wqk = np.empty((E, 512), np.float32)
        wqk[:, :256] = W_q[:, hsl] * scale
        wqk[:, 256:] = W_k[:, hsl]
        bqk = np.empty((512, 1), np.float32)
        bqk[:256, 0] = b_q[hsl] * scale
        bqk[256:, 0] = b_k[hsl]
        wv = np.zeros((E, HL * HV), np.float32)
        bv = np.zeros((1, HL * HV), np.float32)
        for j in range(HL):
            js = slice(4 * g * D + j * D, 4 * g * D + (j + 1) * D)
            wv[:, j * HV:j * HV + 64] = W_v[:, js]
            bv[0, j * HV:j * HV + 64] = b_v[js]
            bv[0, j * HV + 64] = 1.0
        wo = np.ascontiguousarray(W_out[hsl, :])
        bo = (b_out if g == 0 else np.zeros_like(b_out)).reshape(1, E)
        in_maps.append({
            "xT": np.ascontiguousarray(query[b].T).astype(ml_dtypes.bfloat16),
            "wqk": wqk.astype(ml_dtypes.bfloat16),
            "bqk": bqk,
            "wv": wv.astype(ml_dtypes.bfloat16),
            "bv": bv,
            "wo": wo.astype(ml_dtypes.bfloat16),
            "bo": np.ascontiguousarray(bo, dtype=np.float32),
            "nb3": np.full((128, 1), -3.0, np.float32),
            "ones": np.ones((1, 64), np.float32),
        })
    return in_maps


def kernel(query, W_qkv, b_qkv, W_out, b_out):
    if "nc" not in _CACHE:
        _CACHE["nc"] = _build()
    nc = _CACHE["nc"]
    in_maps = _shard_inputs(query, W_qkv, b_qkv, W_out, b_out)
    res = run_bass_kernel_spmd(nc, in_maps, list(range(8)))
    out = np.zeros((2, S, E), np.float32)
    for c in range(8):
        out[c // 4] += np.asarray(res.results[c]["y"]).astype(np.float32)
    return out
